# revision 28
# baseline (speedup 1.0000x reference)
"""Trainium2 Bass kernel for nn_GTCNN (product-graph GTCNN, 2 layers, K collapsed).

Math (per batch b, x: [M=8192, 32]):
  Adj = s0*I + s1*kron(I_t, As) + s2*kron(At, I_s) + s3*kron(At, As),  T=64, N=128
  h0 = x @ W1 + b1 ; h_{l+1} = tanh((Adj h_l) @ Heff_l) ; out = h2 @ W2 + b2

Device decomposition (mixes commute: At on t, P/Q on n, Heff on h):
  P = s0*I + s1*As, Q = s2*I + s3*As (symmetric, folded on host)
  L1 (Heff1 commuted into W1): h0' = x @ (W1 Heff1) + b1 Heff1
      z1 = tanh(P h0' + Q (At-mix h0'))
  L2: u2 = At-mix z1 ; z2 = tanh((P z1 + Q u2) @ Heff2) ; out = z2 W2 + b2

Sharding: core c -> (b = c//4, t-quarter q = c%4). L1 computed fully per b.
All data bf16 (PSUM fp32). t = 32c + tl, n = 32nh + nl, h = feature.
Layouts (mem free-order innermost-last); conversions are DVE stream-transposes
(32x32 partition-inner <-> view-inner-32 swaps, baseline-proven at ~859ns/512):
  x/h0_fd  [(nh,h); (c, tl, nl)]
  g1/z1fdt [(nh,tl); (c, h, nl)]   <- stream swap h<->tl (strided views)
  z_nm     [(nh,nl); (c, tl, h)]   <- stream swap h<->nl
  u1ps     [(nh,tl'); (c', h, nl)] -> stream swap tl'<->nl -> u1_nm [(nh,nl); (c',h,tl')]
  u2ps     [(nh,tl'16); (h, nl)]   -> u2_nm [(nh,nl); (h, tl'32: hi half garbage)]
  zf2/out  [(nh,h); (tl', nl)]
"""

import numpy as np

T, NS, B, FIN, HID, FOUT = 64, 128, 2, 32, 32, 16
M = T * NS
NCORES, NQ = 8, 4
TQ = T // NQ

_CACHE = {}

# const blob column offsets (bf16 [128, 1408])
_CO = dict(w1h=0, atbd=128, pq=640, atbq=896, hi2=1152, w2i4=1280)


def _build_nc():
    from contextlib import ExitStack

    import concourse.mybir as mybir
    import concourse.tile as tile
    from concourse import bacc
    from concourse.bass import ds

    fp = mybir.dt.float32
    bf = mybir.dt.bfloat16
    AF = mybir.ActivationFunctionType

    nc = bacc.Bacc(
        "TRN2",
        target_bir_lowering=False,
        debug=False,
        enable_asserts=False,
        num_devices=NCORES,
    )

    xb = nc.dram_tensor("xb", [128, 2048], bf, kind="ExternalInput")
    cblob = nc.dram_tensor("cblob", [128, 1408], bf, kind="ExternalInput")
    bias2 = nc.dram_tensor("bias2", [128, 2], fp, kind="ExternalInput")
    outb = nc.dram_tensor("outb", [128, 512], bf, kind="ExternalOutput")

    C512 = [slice(512 * j, 512 * (j + 1)) for j in range(4)]
    C1024 = [slice(1024 * j, 1024 * (j + 1)) for j in range(2)]

    with tile.TileContext(nc) as tc, ExitStack() as ctx:
        const = ctx.enter_context(tc.tile_pool(name="const", bufs=1))
        st = ctx.enter_context(tc.tile_pool(name="st", bufs=1))
        ps = ctx.enter_context(tc.tile_pool(name="ps", bufs=2, space="PSUM"))

        pid = nc.partition_id()
        toff = (pid % NQ) * 512  # z1_nm free-offset of this core's t-quarter

        cs = const.tile([128, 1408], bf, tag="cblob")
        nc.sync.dma_start(cs[:], cblob.ap())
        bs = const.tile([128, 2], fp, tag="bias2")
        nc.sync.dma_start(bs[:], bias2.ap())

        def cslice(name, k=0):
            o = _CO[name] + 128 * k
            return cs[:, o : o + 128]

        b1h_s = bs[:, 0:1]
        b2t_s = bs[:, 1:2]

        x_s = st.tile([128, 2048], bf, tag="x")
        for j in range(2):
            nc.scalar.dma_start(x_s[:, C1024[j]], xb.ap()[:, C1024[j]])

        # ---- h0' = x @ W1' + b1' : psum/act in (c, tl, nl) order ----
        h0ps = ps.tile([128, 2048], fp, tag="big")
        for j in range(4):
            nc.tensor.matmul(h0ps[:, C512[j]], cslice("w1h"), x_s[:, C512[j]], start=True, stop=True)
        h0_fd = st.tile([128, 2048], bf, tag="h0_fd")
        for j in range(2):
            nc.scalar.activation(h0_fd[:, C1024[j]], h0ps[:, C1024[j]], AF.Identity, bias=b1h_s)

        # ---- g1 first (slow strided group gates the u1 critical path) ----
        g1 = st.tile([128, 2048], bf, tag="g1")
        h0_t = h0_fd[:].rearrange("p (c tl nl) -> p c nl tl", c=2, tl=32, nl=32)
        g1_v = g1[:].rearrange("p (c h nl) -> p c nl h", c=2, h=32, nl=32)
        for c in range(2):
            for k in range(2):
                sl = slice(16 * k, 16 * (k + 1))
                nc.vector.transpose(out=g1_v[:, c, sl], in_=h0_t[:, c, sl])

        # ---- z_nm: stream swap h<->nl -> [(nh,nl); (c, tl, h)] ----
        z_nm = st.tile([128, 2048], bf, tag="z_nm")
        h0_n = h0_fd[:].rearrange("p (c tl nl) -> p c tl nl", c=2, tl=32, nl=32)
        z_nm_v = z_nm[:].rearrange("p (c tl h) -> p c tl h", c=2, tl=32, h=32)
        for c in range(2):
            for k in range(2):
                sl = slice(16 * k, 16 * (k + 1))
                nc.vector.transpose(out=z_nm_v[:, c, sl], in_=h0_n[:, c, sl])

        # ---- zpre1 P-part (overlaps u1 chain); psum order (c, h, tl) ----
        zpre1 = ps.tile([128, 2048], fp, tag="big")
        z_nm_q = z_nm[:].rearrange("p (c tl h) -> p c h tl", c=2, tl=32, h=32)
        for j in range(4):
            mv = z_nm_q[:, j // 2, 16 * (j % 2) : 16 * (j % 2) + 16, :]
            nc.tensor.matmul(zpre1[:, C512[j]], cslice("pq", 0), mv, start=True, stop=False)

        # ---- u1 = At-mix(h0') : psum [(nh,tl'); (c', h, nl)] ----
        u1ps = ps.tile([128, 2048], fp, tag="big")
        for cp in range(2):
            for hh in range(2):
                dst = u1ps[:, 1024 * cp + 512 * hh : 1024 * cp + 512 * (hh + 1)]
                for c in range(2):
                    nc.tensor.matmul(
                        dst,
                        cslice("atbd", 2 * c + cp),
                        g1[:, 1024 * c + 512 * hh : 1024 * c + 512 * (hh + 1)],
                        start=(c == 0),
                        stop=(c == 1),
                    )

        # ---- u1 -> NM: ScalarE psum->bf16 copy, then stream swap tl'<->nl ----
        u1c = st.tile([128, 2048], bf, tag="u1c")
        for j in range(2):
            nc.scalar.copy(u1c[:, C1024[j]], u1ps[:, C1024[j]])
        u1_nm = st.tile([128, 2048], bf, tag="u1_nm")
        u1c_v = u1c[:].rearrange("p (c h nl) -> p c h nl", c=2, h=32, nl=32)
        u1_nm_v = u1_nm[:].rearrange("p (c h tl) -> p c h tl", c=2, h=32, tl=32)
        for c in range(2):
            for k in range(2):
                sl = slice(16 * k, 16 * (k + 1))
                nc.vector.transpose(out=u1_nm_v[:, c, sl], in_=u1c_v[:, c, sl])

        # ---- zpre1 += Q u1 (contiguous moving, matching (c, h, tl) order) ----
        for j in range(4):
            nc.tensor.matmul(zpre1[:, C512[j]], cslice("pq", 1), u1_nm[:, C512[j]], start=False, stop=True)

        # ---- tanh -> z1_nm [(nh,nl); (c, h, tl)] (contiguous) ----
        z1_nm = st.tile([128, 2048], bf, tag="z1_nm")
        for j in range(2):
            nc.scalar.activation(z1_nm[:, C1024[j]], zpre1[:, C1024[j]], AF.Tanh)

        # ================== layer 2 (t-quarter only) ==================
        # ---- g2: stream swap nl<->tl -> z1_fdt [(nh,tl); (c, h, nl)] ----
        z1_fdt = st.tile([128, 2048], bf, tag="z1_fdt")
        z1_t = z1_nm[:].rearrange("p (c h tl) -> p c h tl", c=2, h=32, tl=32)
        z1_fdt_v = z1_fdt[:].rearrange("p (c h nl) -> p c h nl", c=2, h=32, nl=32)
        for c in range(2):
            for k in range(2):
                sl = slice(16 * k, 16 * (k + 1))
                nc.vector.transpose(out=z1_fdt_v[:, c, sl], in_=z1_t[:, c, sl])

        # ---- u2 = At-mix(z1) quarter : psum [(nh,tl'16); (h, nl)] ----
        u2ps = ps.tile([128, 1024], fp, tag="big")
        for hh in range(2):
            dst = u2ps[:, 512 * hh : 512 * (hh + 1)]
            for c in range(2):
                nc.tensor.matmul(
                    dst,
                    cslice("atbq", c),
                    z1_fdt[:, 1024 * c + 512 * hh : 1024 * c + 512 * (hh + 1)],
                    start=(c == 0),
                    stop=(c == 1),
                )

        # ---- u2 -> NM: stream swap tl'<->nl -> [(nh,nl); (h, tl'32)] ----
        u2c = st.tile([128, 1024], bf, tag="u2c")
        nc.scalar.copy(u2c[:], u2ps[:])
        u2_nm = st.tile([128, 1024], bf, tag="u2_nm")
        u2c_v = u2c[:].rearrange("p (h nl) -> p h nl", h=32, nl=32)
        u2_nm_v = u2_nm[:].rearrange("p (h tl) -> p h tl", h=32, tl=32)
        for k in range(2):
            sl = slice(16 * k, 16 * (k + 1))
            nc.vector.transpose(out=u2_nm_v[:, sl], in_=u2c_v[:, sl])

        # ---- zpre2: full-width P2 over 4 static quarter-chunks (tl', h) ----
        zpre2 = ps.tile([128, 2048], fp, tag="big")
        z1_tlh = z1_nm[:].rearrange("p (c h tl) -> p c tl h", c=2, h=32, tl=32)
        for j in range(4):
            mv = z1_tlh[:, j // 2, 16 * (j % 2) : 16 * (j % 2) + 16, :]
            nc.tensor.matmul(zpre2[:, C512[j]], cslice("pq", 0), mv, start=True, stop=True)
        zq2 = ps.tile([128, 512], fp, tag="big")
        u2q = u2_nm[:].rearrange("p (h tl) -> p tl h", h=32, tl=32)[:, 0:TQ, :]
        nc.tensor.matmul(zq2[:], cslice("pq", 1), u2q, start=True, stop=True)
        qadd = st.tile([128, 512], bf, tag="qadd")
        nc.vector.tensor_copy(qadd[:], zq2[:])

        # ---- zf2: DVE cast copy + stream swap nl<->h -> [(nh,h); (tl', nl)] ----
        zf2c = st.tile([128, 512], bf, tag="zf2c")
        nc.vector.tensor_add(zf2c[:], zpre2[:, ds(toff, 512)], qadd[:])
        zf2 = st.tile([128, 512], bf, tag="zf2")
        zf2c_v = zf2c[:].rearrange("p (tl h) -> p tl h", tl=TQ)
        zf2_v = zf2[:].rearrange("p (tl nl) -> p tl nl", tl=TQ)
        nc.vector.transpose(out=zf2_v, in_=zf2c_v)

        # ---- pre2 = zf2 @ Heff2 ; tanh ; out = z2 @ W2 + b2 ----
        pre2 = ps.tile([128, 512], fp, tag="big")
        nc.tensor.matmul(pre2[:], cslice("hi2"), zf2[:], start=True, stop=True)
        h2_fd = st.tile([128, 512], bf, tag="h2_fd")
        nc.scalar.activation(h2_fd[:], pre2[:], AF.Tanh)

        opre = ps.tile([128, 512], fp, tag="big")
        nc.tensor.matmul(opre[:], cslice("w2i4"), h2_fd[:], start=True, stop=True)
        out_s = st.tile([128, 512], bf, tag="out_s")
        nc.scalar.activation(out_s[:], opre[:], AF.Identity, bias=b2t_s)

        nc.sync.dma_start(outb.ap(), out_s[:])

    nc.compile()
    return nc


def _host_weights(Adj_t, Adj_s, s, H, W1, b1, W2, b2):
    import ml_dtypes

    f4, bfd = np.float32, ml_dtypes.bfloat16
    I4 = np.eye(4, dtype=f4)
    I128 = np.eye(128, dtype=f4)
    Heff = H.sum(axis=1).astype(f4)

    P = (s[0] * I128 + s[1] * Adj_s).astype(f4)
    Q = (s[2] * I128 + s[3] * Adj_s).astype(f4)

    W1h = (W1 @ Heff[0]).astype(f4)
    b1h_vec = (b1 @ Heff[0]).astype(f4)
    w1h = np.kron(I4, W1h)
    b1h = np.tile(b1h_vec, 4)[:, None].astype(f4)

    hi2 = np.kron(I4, Heff[1])
    w2pad = np.zeros((32, 32), dtype=f4)
    w2pad[:, :FOUT] = W2
    w2i4 = np.kron(I4, w2pad)
    b2pad = np.zeros(32, dtype=f4)
    b2pad[:FOUT] = b2
    b2t = np.tile(b2pad, 4)[:, None].astype(f4)

    atbd = [
        np.kron(I4, Adj_t[32 * c : 32 * (c + 1), 32 * cp : 32 * (cp + 1)].astype(f4))
        for c in range(2)
        for cp in range(2)
    ]  # index 2c+cp

    atbq_all = []
    for qq in range(NQ):
        blks = []
        for c in range(2):
            blk = np.zeros((32, 32), dtype=f4)
            blk[:, :TQ] = Adj_t[32 * c : 32 * (c + 1), TQ * qq : TQ * (qq + 1)]
            blks.append(np.kron(I4, blk))
        atbq_all.append(blks)

    blobs = []
    for qq in range(NQ):
        blob = np.concatenate(
            [w1h] + atbd + [P, Q] + atbq_all[qq] + [hi2, w2i4], axis=1
        ).astype(bfd)
        assert blob.shape == (128, 1408)
        blobs.append(np.ascontiguousarray(blob))
    bias2 = np.concatenate([b1h, b2t], axis=1).astype(f4)
    return blobs, np.ascontiguousarray(bias2)


def _x_fd(xb):
    """x [8192, 32] f32 -> [(nh,f); (c, tl, nl)] bf16 [128, 2048]."""
    import ml_dtypes

    v = xb.reshape(2, 32, 4, 32, 32)  # [c, tl, nh, nl, f]
    v = v.transpose(2, 4, 0, 1, 3)  # [nh, f, c, tl, nl]
    return np.ascontiguousarray(v.reshape(128, 2048).astype(ml_dtypes.bfloat16))


def _in_maps(inputs):
    f4 = np.float32
    x = np.asarray(inputs["x"], dtype=f4)
    blobs, bias2 = _host_weights(
        np.asarray(inputs["Adj_t"], dtype=f4),
        np.asarray(inputs["Adj_s"], dtype=f4),
        np.asarray(inputs["s"], dtype=f4),
        np.asarray(inputs["H"], dtype=f4),
        np.asarray(inputs["W1"], dtype=f4),
        np.asarray(inputs["b1"], dtype=f4),
        np.asarray(inputs["W2"], dtype=f4),
        np.asarray(inputs["b2"], dtype=f4),
    )
    xfd = [_x_fd(x[b]) for b in range(B)]
    maps = []
    for c in range(NCORES):
        b, qq = c // NQ, c % NQ
        maps.append({"xb": xfd[b], "cblob": blobs[qq], "bias2": bias2})
    return maps


def _gather_out(res_list):
    """Per-core outb [128, 512] bf16 ([(nh,f); (tl', nl)]) -> [B, M, FOUT] f32."""
    out = np.empty((B, M, FOUT), dtype=np.float32)
    for c in range(NCORES):
        b, qq = c // NQ, c % NQ
        o = np.asarray(res_list[c]["outb"]).astype(np.float32)
        v = o.reshape(4, 32, TQ, 32)  # [nh, f, tl, nl]
        v = v.transpose(2, 0, 3, 1).reshape(TQ, 128, 32)  # [tl, n, f]
        tbase = TQ * qq
        out[b, 128 * tbase : 128 * (tbase + TQ), :] = v[:, :, :FOUT].reshape(
            TQ * 128, FOUT
        )
    return out


def kernel(**inputs) -> np.ndarray:
    from concourse import bass_utils

    if "nc" not in _CACHE:
        _CACHE["nc"] = _build_nc()
    nc = _CACHE["nc"]

    maps = _in_maps(inputs)
    import os

    trace = bool(int(os.environ.get("GTCNN_TRACE", "0")))
    res = bass_utils.run_bass_kernel_spmd(
        nc,
        maps,
        core_ids=list(range(NCORES)),
        trace=trace,
        trace_cores=list(range(NCORES)) if trace else None,
        stitch_traces=False,
    )
    _CACHE["last_results"] = res

    return _gather_out(res.results)


# revision 29
# speedup vs baseline: 1.0197x; 1.0197x over previous
"""Trainium2 Bass kernel for nn_GTCNN (product-graph GTCNN, 2 layers, K collapsed).

Math (per batch b, x: [M=8192, 32]):
  Adj = s0*I + s1*kron(I_t, As) + s2*kron(At, I_s) + s3*kron(At, As),  T=64, N=128
  h0 = x @ W1 + b1 ; h_{l+1} = tanh((Adj h_l) @ Heff_l) ; out = h2 @ W2 + b2

Device decomposition (mixes commute: At on t, P/Q on n, Heff on h):
  P = s0*I + s1*As, Q = s2*I + s3*As (symmetric, folded on host)
  L1 (Heff1 commuted into W1): h0' = x @ (W1 Heff1) + b1 Heff1
      z1 = tanh(P h0' + Q (At-mix h0'))
  L2: u2 = At-mix z1 ; z2 = tanh((P z1 + Q u2) @ Heff2) ; out = z2 W2 + b2

Sharding: core c -> (b = c//4, t-quarter q = c%4). L1 computed fully per b.
All data bf16 (PSUM fp32). t = 32c + tl, n = 32nh + nl, h = feature.
Layouts (mem free-order innermost-last); conversions are DVE stream-transposes
(32x32 partition-inner <-> view-inner-32 swaps, baseline-proven at ~859ns/512):
  x/h0_fd  [(nh,h); (c, tl, nl)]
  g1/z1fdt [(nh,tl); (c, h, nl)]   <- stream swap h<->tl (strided views)
  z_nm     [(nh,nl); (c, tl, h)]   <- stream swap h<->nl
  u1ps     [(nh,tl'); (c', h, nl)] -> stream swap tl'<->nl -> u1_nm [(nh,nl); (c',h,tl')]
  u2ps     [(nh,tl'16); (h, nl)]   -> u2_nm [(nh,nl); (h, tl'32: hi half garbage)]
  zf2/out  [(nh,h); (tl', nl)]
"""

import numpy as np

T, NS, B, FIN, HID, FOUT = 64, 128, 2, 32, 32, 16
M = T * NS
NCORES, NQ = 8, 4
TQ = T // NQ

_CACHE = {}

# const blob column offsets (bf16 [128, 1408])
_CO = dict(w1h=0, atbd=128, pq=640, atbq=896, hi2=1152, w2i4=1280)


def _build_nc():
    from contextlib import ExitStack

    import concourse.mybir as mybir
    import concourse.tile as tile
    from concourse import bacc
    from concourse.bass import ds

    fp = mybir.dt.float32
    bf = mybir.dt.bfloat16
    AF = mybir.ActivationFunctionType

    nc = bacc.Bacc(
        "TRN2",
        target_bir_lowering=False,
        debug=False,
        enable_asserts=False,
        num_devices=NCORES,
    )

    xb = nc.dram_tensor("xb", [128, 2048], bf, kind="ExternalInput")
    cblob = nc.dram_tensor("cblob", [128, 1408], bf, kind="ExternalInput")
    bias2 = nc.dram_tensor("bias2", [128, 2], fp, kind="ExternalInput")
    outb = nc.dram_tensor("outb", [128, 512], bf, kind="ExternalOutput")

    C512 = [slice(512 * j, 512 * (j + 1)) for j in range(4)]
    C1024 = [slice(1024 * j, 1024 * (j + 1)) for j in range(2)]

    with tile.TileContext(nc) as tc, ExitStack() as ctx:
        const = ctx.enter_context(tc.tile_pool(name="const", bufs=1))
        st = ctx.enter_context(tc.tile_pool(name="st", bufs=1))
        ps = ctx.enter_context(tc.tile_pool(name="ps", bufs=2, space="PSUM"))

        pid = nc.partition_id()
        q = pid % NQ
        cq = q // 2          # t-half of this core's quarter
        tl0 = (q % 2) * TQ   # tl offset within the half

        cs = const.tile([128, 1408], bf, tag="cblob")
        nc.sync.dma_start(cs[:], cblob.ap())
        bs = const.tile([128, 2], fp, tag="bias2")
        nc.sync.dma_start(bs[:], bias2.ap())

        def cslice(name, k=0):
            o = _CO[name] + 128 * k
            return cs[:, o : o + 128]

        b1h_s = bs[:, 0:1]
        b2t_s = bs[:, 1:2]

        x_s = st.tile([128, 2048], bf, tag="x")
        for j in range(2):
            nc.scalar.dma_start(x_s[:, C1024[j]], xb.ap()[:, C1024[j]])

        # ---- h0' = x @ W1' + b1' : psum/act in (c, tl, nl) order ----
        h0ps = ps.tile([128, 2048], fp, tag="big")
        for j in range(4):
            nc.tensor.matmul(h0ps[:, C512[j]], cslice("w1h"), x_s[:, C512[j]], start=True, stop=True)
        h0_fd = st.tile([128, 2048], bf, tag="h0_fd")
        for j in range(2):
            nc.scalar.activation(h0_fd[:, C1024[j]], h0ps[:, C1024[j]], AF.Identity, bias=b1h_s)

        # ---- g1 first (slow strided group gates the u1 critical path) ----
        g1 = st.tile([128, 2048], bf, tag="g1")
        h0_t = h0_fd[:].rearrange("p (c tl nl) -> p c nl tl", c=2, tl=32, nl=32)
        g1_v = g1[:].rearrange("p (c h nl) -> p c nl h", c=2, h=32, nl=32)
        for c in range(2):
            for k in range(2):
                sl = slice(16 * k, 16 * (k + 1))
                nc.vector.transpose(out=g1_v[:, c, sl], in_=h0_t[:, c, sl])

        # ---- z_nm: stream swap h<->nl -> [(nh,nl); (c, tl, h)] ----
        z_nm = st.tile([128, 2048], bf, tag="z_nm")
        h0_n = h0_fd[:].rearrange("p (c tl nl) -> p c tl nl", c=2, tl=32, nl=32)
        z_nm_v = z_nm[:].rearrange("p (c tl h) -> p c tl h", c=2, tl=32, h=32)
        for c in range(2):
            for k in range(2):
                sl = slice(16 * k, 16 * (k + 1))
                nc.vector.transpose(out=z_nm_v[:, c, sl], in_=h0_n[:, c, sl])

        # ---- zpre1 P-part (overlaps u1 chain); psum order (c, h, tl) ----
        zpre1 = ps.tile([128, 2048], fp, tag="big")
        z_nm_q = z_nm[:].rearrange("p (c tl h) -> p c h tl", c=2, tl=32, h=32)
        for j in range(4):
            mv = z_nm_q[:, j // 2, 16 * (j % 2) : 16 * (j % 2) + 16, :]
            nc.tensor.matmul(zpre1[:, C512[j]], cslice("pq", 0), mv, start=True, stop=False)

        # ---- u1 = At-mix(h0') : psum [(nh,tl'); (c', h, nl)] ----
        u1ps = ps.tile([128, 2048], fp, tag="big")
        for cp in range(2):
            for hh in range(2):
                dst = u1ps[:, 1024 * cp + 512 * hh : 1024 * cp + 512 * (hh + 1)]
                for c in range(2):
                    nc.tensor.matmul(
                        dst,
                        cslice("atbd", 2 * c + cp),
                        g1[:, 1024 * c + 512 * hh : 1024 * c + 512 * (hh + 1)],
                        start=(c == 0),
                        stop=(c == 1),
                    )

        # ---- u1 -> NM: ScalarE psum->bf16 copy, then stream swap tl'<->nl ----
        u1c = st.tile([128, 2048], bf, tag="u1c")
        for j in range(2):
            nc.scalar.copy(u1c[:, C1024[j]], u1ps[:, C1024[j]])
        u1_nm = st.tile([128, 2048], bf, tag="u1_nm")
        u1c_v = u1c[:].rearrange("p (c h nl) -> p c h nl", c=2, h=32, nl=32)
        u1_nm_v = u1_nm[:].rearrange("p (c h tl) -> p c h tl", c=2, h=32, tl=32)
        for c in range(2):
            for k in range(2):
                sl = slice(16 * k, 16 * (k + 1))
                nc.vector.transpose(out=u1_nm_v[:, c, sl], in_=u1c_v[:, c, sl])

        # ---- zpre1 += Q u1 (contiguous moving, matching (c, h, tl) order) ----
        for j in range(4):
            nc.tensor.matmul(zpre1[:, C512[j]], cslice("pq", 1), u1_nm[:, C512[j]], start=False, stop=True)

        # ---- tanh -> z1_nm [(nh,nl); (c, h, tl)] (contiguous) ----
        z1_nm = st.tile([128, 2048], bf, tag="z1_nm")
        for j in range(2):
            nc.scalar.activation(z1_nm[:, C1024[j]], zpre1[:, C1024[j]], AF.Tanh)

        # ================== layer 2 (t-quarter only) ==================
        # ---- g2: stream swap nl<->tl -> z1_fdt [(nh,tl); (c, h, nl)] ----
        z1_fdt = st.tile([128, 2048], bf, tag="z1_fdt")
        z1_t = z1_nm[:].rearrange("p (c h tl) -> p c h tl", c=2, h=32, tl=32)
        z1_fdt_v = z1_fdt[:].rearrange("p (c h nl) -> p c h nl", c=2, h=32, nl=32)
        for c in range(2):
            for k in range(2):
                sl = slice(16 * k, 16 * (k + 1))
                nc.vector.transpose(out=z1_fdt_v[:, c, sl], in_=z1_t[:, c, sl])

        # ---- u2 = At-mix(z1) quarter : psum [(nh,tl'16); (h, nl)] ----
        u2ps = ps.tile([128, 1024], fp, tag="big")
        for hh in range(2):
            dst = u2ps[:, 512 * hh : 512 * (hh + 1)]
            for c in range(2):
                nc.tensor.matmul(
                    dst,
                    cslice("atbq", c),
                    z1_fdt[:, 1024 * c + 512 * hh : 1024 * c + 512 * (hh + 1)],
                    start=(c == 0),
                    stop=(c == 1),
                )

        # ---- u2 -> NM: stream swap tl'<->nl -> [(nh,nl); (h, tl'32)] ----
        u2c = st.tile([128, 1024], bf, tag="u2c")
        nc.scalar.copy(u2c[:], u2ps[:])
        u2_nm = st.tile([128, 1024], bf, tag="u2_nm")
        u2c_v = u2c[:].rearrange("p (h nl) -> p h nl", h=32, nl=32)
        u2_nm_v = u2_nm[:].rearrange("p (h tl) -> p h tl", h=32, tl=32)
        for k in range(2):
            sl = slice(16 * k, 16 * (k + 1))
            nc.vector.transpose(out=u2_nm_v[:, sl], in_=u2c_v[:, sl])

        # ---- zpre2 = P z1q + Q u2 : psum [(n'); (tl', h)], dynamic quarter ----
        zpre2 = ps.tile([128, 512], fp, tag="big")
        z1_tlh = z1_nm[:].rearrange("p (c h tl) -> p c tl h", c=2, h=32, tl=32)
        z1q = z1_tlh[:, ds(cq, 1), ds(tl0, TQ), :]
        nc.tensor.matmul(zpre2[:], cslice("pq", 0), z1q, start=True, stop=False)
        u2q = u2_nm[:].rearrange("p (h tl) -> p tl h", h=32, tl=32)[:, 0:TQ, :]
        nc.tensor.matmul(zpre2[:], cslice("pq", 1), u2q, start=False, stop=True)

        # ---- zf2: DVE cast copy + stream swap nl<->h -> [(nh,h); (tl', nl)] ----
        zf2c = st.tile([128, 512], bf, tag="zf2c")
        nc.vector.tensor_copy(zf2c[:], zpre2[:])
        zf2 = st.tile([128, 512], bf, tag="zf2")
        zf2c_v = zf2c[:].rearrange("p (tl h) -> p tl h", tl=TQ)
        zf2_v = zf2[:].rearrange("p (tl nl) -> p tl nl", tl=TQ)
        nc.vector.transpose(out=zf2_v, in_=zf2c_v)

        # ---- pre2 = zf2 @ Heff2 ; tanh ; out = z2 @ W2 + b2 ----
        pre2 = ps.tile([128, 512], fp, tag="big")
        nc.tensor.matmul(pre2[:], cslice("hi2"), zf2[:], start=True, stop=True)
        h2_fd = st.tile([128, 512], bf, tag="h2_fd")
        nc.scalar.activation(h2_fd[:], pre2[:], AF.Tanh)

        opre = ps.tile([128, 512], fp, tag="big")
        nc.tensor.matmul(opre[:], cslice("w2i4"), h2_fd[:], start=True, stop=True)
        out_s = st.tile([128, 512], bf, tag="out_s")
        nc.scalar.activation(out_s[:], opre[:], AF.Identity, bias=b2t_s)

        nc.sync.dma_start(outb.ap(), out_s[:])

    nc.compile()
    return nc


def _host_weights(Adj_t, Adj_s, s, H, W1, b1, W2, b2):
    import ml_dtypes

    f4, bfd = np.float32, ml_dtypes.bfloat16
    I4 = np.eye(4, dtype=f4)
    I128 = np.eye(128, dtype=f4)
    Heff = H.sum(axis=1).astype(f4)

    P = (s[0] * I128 + s[1] * Adj_s).astype(f4)
    Q = (s[2] * I128 + s[3] * Adj_s).astype(f4)

    W1h = (W1 @ Heff[0]).astype(f4)
    b1h_vec = (b1 @ Heff[0]).astype(f4)
    w1h = np.kron(I4, W1h)
    b1h = np.tile(b1h_vec, 4)[:, None].astype(f4)

    hi2 = np.kron(I4, Heff[1])
    w2pad = np.zeros((32, 32), dtype=f4)
    w2pad[:, :FOUT] = W2
    w2i4 = np.kron(I4, w2pad)
    b2pad = np.zeros(32, dtype=f4)
    b2pad[:FOUT] = b2
    b2t = np.tile(b2pad, 4)[:, None].astype(f4)

    atbd = [
        np.kron(I4, Adj_t[32 * c : 32 * (c + 1), 32 * cp : 32 * (cp + 1)].astype(f4))
        for c in range(2)
        for cp in range(2)
    ]  # index 2c+cp

    atbq_all = []
    for qq in range(NQ):
        blks = []
        for c in range(2):
            blk = np.zeros((32, 32), dtype=f4)
            blk[:, :TQ] = Adj_t[32 * c : 32 * (c + 1), TQ * qq : TQ * (qq + 1)]
            blks.append(np.kron(I4, blk))
        atbq_all.append(blks)

    blobs = []
    for qq in range(NQ):
        blob = np.concatenate(
            [w1h] + atbd + [P, Q] + atbq_all[qq] + [hi2, w2i4], axis=1
        ).astype(bfd)
        assert blob.shape == (128, 1408)
        blobs.append(np.ascontiguousarray(blob))
    bias2 = np.concatenate([b1h, b2t], axis=1).astype(f4)
    return blobs, np.ascontiguousarray(bias2)


def _x_fd(xb):
    """x [8192, 32] f32 -> [(nh,f); (c, tl, nl)] bf16 [128, 2048]."""
    import ml_dtypes

    v = xb.reshape(2, 32, 4, 32, 32)  # [c, tl, nh, nl, f]
    v = v.transpose(2, 4, 0, 1, 3)  # [nh, f, c, tl, nl]
    return np.ascontiguousarray(v.reshape(128, 2048).astype(ml_dtypes.bfloat16))


def _in_maps(inputs):
    f4 = np.float32
    x = np.asarray(inputs["x"], dtype=f4)
    blobs, bias2 = _host_weights(
        np.asarray(inputs["Adj_t"], dtype=f4),
        np.asarray(inputs["Adj_s"], dtype=f4),
        np.asarray(inputs["s"], dtype=f4),
        np.asarray(inputs["H"], dtype=f4),
        np.asarray(inputs["W1"], dtype=f4),
        np.asarray(inputs["b1"], dtype=f4),
        np.asarray(inputs["W2"], dtype=f4),
        np.asarray(inputs["b2"], dtype=f4),
    )
    xfd = [_x_fd(x[b]) for b in range(B)]
    maps = []
    for c in range(NCORES):
        b, qq = c // NQ, c % NQ
        maps.append({"xb": xfd[b], "cblob": blobs[qq], "bias2": bias2})
    return maps


def _gather_out(res_list):
    """Per-core outb [128, 512] bf16 ([(nh,f); (tl', nl)]) -> [B, M, FOUT] f32."""
    out = np.empty((B, M, FOUT), dtype=np.float32)
    for c in range(NCORES):
        b, qq = c // NQ, c % NQ
        o = np.asarray(res_list[c]["outb"]).astype(np.float32)
        v = o.reshape(4, 32, TQ, 32)  # [nh, f, tl, nl]
        v = v.transpose(2, 0, 3, 1).reshape(TQ, 128, 32)  # [tl, n, f]
        tbase = TQ * qq
        out[b, 128 * tbase : 128 * (tbase + TQ), :] = v[:, :, :FOUT].reshape(
            TQ * 128, FOUT
        )
    return out


def kernel(**inputs) -> np.ndarray:
    from concourse import bass_utils

    if "nc" not in _CACHE:
        _CACHE["nc"] = _build_nc()
    nc = _CACHE["nc"]

    maps = _in_maps(inputs)
    import os

    trace = bool(int(os.environ.get("GTCNN_TRACE", "0")))
    res = bass_utils.run_bass_kernel_spmd(
        nc,
        maps,
        core_ids=list(range(NCORES)),
        trace=trace,
        trace_cores=list(range(NCORES)) if trace else None,
        stitch_traces=False,
    )
    _CACHE["last_results"] = res

    return _gather_out(res.results)


# revision 30
# speedup vs baseline: 1.0568x; 1.0364x over previous
"""Trainium2 Bass kernel for nn_GTCNN (product-graph GTCNN, 2 layers, K collapsed).

Math (per batch b, x: [M=8192, 32]):
  Adj = s0*I + s1*kron(I_t, As) + s2*kron(At, I_s) + s3*kron(At, As),  T=64, N=128
  h0 = x @ W1 + b1 ; h_{l+1} = tanh((Adj h_l) @ Heff_l) ; out = h2 @ W2 + b2

Device decomposition (mixes commute: At on t, P/Q on n, Heff on h):
  P = s0*I + s1*As, Q = s2*I + s3*As (symmetric, folded on host)
  L1 (Heff1 commuted into W1): h0' = x @ (W1 Heff1) + b1 Heff1
      z1 = tanh(P h0' + Q (At-mix h0'))
  L2: u2 = At-mix z1 ; z2 = tanh((P z1 + Q u2) @ Heff2) ; out = z2 W2 + b2

Sharding: core c -> (b = c//4, t-quarter q = c%4). L1 computed fully per b.
All data bf16 (PSUM fp32). t = 32c + tl, n = 32nh + nl, h = feature.
Layouts (mem free-order innermost-last); conversions are DVE stream-transposes
(32x32 partition-inner <-> view-inner-32 swaps, baseline-proven at ~859ns/512):
  x/h0_fd  [(nh,h); (c, tl, nl)]
  g1/z1fdt [(nh,tl); (c, h, nl)]   <- stream swap h<->tl (strided views)
  z_nm     [(nh,nl); (c, tl, h)]   <- stream swap h<->nl
  u1ps     [(nh,tl'); (c', h, nl)] -> stream swap tl'<->nl -> u1_nm [(nh,nl); (c',h,tl')]
  u2ps     [(nh,tl'16); (h, nl)]   -> u2_nm [(nh,nl); (h, tl'32: hi half garbage)]
  zf2/out  [(nh,h); (tl', nl)]
"""

import numpy as np

T, NS, B, FIN, HID, FOUT = 64, 128, 2, 32, 32, 16
M = T * NS
NCORES, NQ = 8, 4
TQ = T // NQ

_CACHE = {}

# const blob column offsets (bf16 [128, 1408])
_CO = dict(w1h=0, atbd=128, pq=640, atbq=896, hi2=1152, w2i4=1280)


def _build_nc():
    from contextlib import ExitStack

    import concourse.mybir as mybir
    import concourse.tile as tile
    from concourse import bacc
    from concourse.bass import ds

    fp = mybir.dt.float32
    bf = mybir.dt.bfloat16
    AF = mybir.ActivationFunctionType

    nc = bacc.Bacc(
        "TRN2",
        target_bir_lowering=False,
        debug=False,
        enable_asserts=False,
        num_devices=NCORES,
    )

    xb = nc.dram_tensor("xb", [128, 2048], bf, kind="ExternalInput")
    cblob = nc.dram_tensor("cblob", [128, 1408], bf, kind="ExternalInput")
    bias2 = nc.dram_tensor("bias2", [128, 2], fp, kind="ExternalInput")
    outb = nc.dram_tensor("outb", [128, 512], bf, kind="ExternalOutput")

    C512 = [slice(512 * j, 512 * (j + 1)) for j in range(4)]
    C1024 = [slice(1024 * j, 1024 * (j + 1)) for j in range(2)]

    with tile.TileContext(nc) as tc, ExitStack() as ctx:
        const = ctx.enter_context(tc.tile_pool(name="const", bufs=1))
        st = ctx.enter_context(tc.tile_pool(name="st", bufs=1))
        ps = ctx.enter_context(tc.tile_pool(name="ps", bufs=2, space="PSUM"))

        pid = nc.partition_id()
        q = pid % NQ
        cq = q // 2          # t-half of this core's quarter
        tl0 = (q % 2) * TQ   # tl offset within the half

        cs = const.tile([128, 1408], bf, tag="cblob")
        nc.sync.dma_start(cs[:], cblob.ap())
        bs = const.tile([128, 2], fp, tag="bias2")
        nc.sync.dma_start(bs[:], bias2.ap())

        def cslice(name, k=0):
            o = _CO[name] + 128 * k
            return cs[:, o : o + 128]

        b1h_s = bs[:, 0:1]
        b2t_s = bs[:, 1:2]

        x_s = st.tile([128, 2048], bf, tag="x")
        for j in range(2):
            nc.scalar.dma_start(x_s[:, C1024[j]], xb.ap()[:, C1024[j]])

        # ---- h0' = x @ W1' + b1' : psum/act in (c, tl, nl) order ----
        h0ps = ps.tile([128, 2048], fp, tag="big")
        for j in range(4):
            nc.tensor.matmul(h0ps[:, C512[j]], cslice("w1h"), x_s[:, C512[j]], start=True, stop=True)
        h0_fd = st.tile([128, 2048], bf, tag="h0_fd")
        for j in range(2):
            nc.scalar.activation(h0_fd[:, C1024[j]], h0ps[:, C1024[j]], AF.Identity, bias=b1h_s)

        # ---- g1 first (slow strided group gates the u1 critical path) ----
        g1 = st.tile([128, 2048], bf, tag="g1")
        h0_t = h0_fd[:].rearrange("p (c tl nl) -> p c nl tl", c=2, tl=32, nl=32)
        g1_v = g1[:].rearrange("p (c h nl) -> p c nl h", c=2, h=32, nl=32)
        for c in range(2):
            for k in range(2):
                sl = slice(16 * k, 16 * (k + 1))
                nc.vector.transpose(out=g1_v[:, c, sl], in_=h0_t[:, c, sl])

        # ---- z_nm: stream swap h<->nl -> [(nh,nl); (c, tl, h)] ----
        z_nm = st.tile([128, 2048], bf, tag="z_nm")
        h0_n = h0_fd[:].rearrange("p (c tl nl) -> p c tl nl", c=2, tl=32, nl=32)
        z_nm_v = z_nm[:].rearrange("p (c tl h) -> p c tl h", c=2, tl=32, h=32)
        for c in range(2):
            for k in range(2):
                sl = slice(16 * k, 16 * (k + 1))
                nc.vector.transpose(out=z_nm_v[:, c, sl], in_=h0_n[:, c, sl])

        # ---- zpre1 psum tile (order (c, h, tl)); mms emitted after u1 ----
        zpre1 = ps.tile([128, 2048], fp, tag="big")
        z_nm_q = z_nm[:].rearrange("p (c tl h) -> p c h tl", c=2, tl=32, h=32)

        # ---- u1 = At-mix(h0') : psum [(nh,tl'); (c', h, nl)] ----
        u1ps = ps.tile([128, 2048], fp, tag="big")
        for cp in range(2):
            for hh in range(2):
                dst = u1ps[:, 1024 * cp + 512 * hh : 1024 * cp + 512 * (hh + 1)]
                for c in range(2):
                    nc.tensor.matmul(
                        dst,
                        cslice("atbd", 2 * c + cp),
                        g1[:, 1024 * c + 512 * hh : 1024 * c + 512 * (hh + 1)],
                        start=(c == 0),
                        stop=(c == 1),
                    )

        # ---- u1 -> NM: ScalarE psum->bf16 copy, then stream swap tl'<->nl ----
        u1c = st.tile([128, 2048], bf, tag="u1c")
        for j in range(2):
            nc.scalar.copy(u1c[:, C1024[j]], u1ps[:, C1024[j]])
        u1_nm = st.tile([128, 2048], bf, tag="u1_nm")
        u1c_v = u1c[:].rearrange("p (c h nl) -> p c h nl", c=2, h=32, nl=32)
        u1_nm_v = u1_nm[:].rearrange("p (c h tl) -> p c h tl", c=2, h=32, tl=32)
        for c in range(2):
            for k in range(2):
                sl = slice(16 * k, 16 * (k + 1))
                nc.vector.transpose(out=u1_nm_v[:, c, sl], in_=u1c_v[:, c, sl])

        # ---- zpre1 = Q u1 + P z (Q first; strided P overlaps u1c/streams) ----
        for j in range(4):
            nc.tensor.matmul(zpre1[:, C512[j]], cslice("pq", 1), u1_nm[:, C512[j]], start=True, stop=False)
        for j in range(4):
            mv = z_nm_q[:, j // 2, 16 * (j % 2) : 16 * (j % 2) + 16, :]
            nc.tensor.matmul(zpre1[:, C512[j]], cslice("pq", 0), mv, start=False, stop=True)

        # ---- tanh -> z1_nm [(nh,nl); (c, h, tl)] (contiguous) ----
        z1_nm = st.tile([128, 2048], bf, tag="z1_nm")
        for j in range(2):
            nc.scalar.activation(z1_nm[:, C1024[j]], zpre1[:, C1024[j]], AF.Tanh)

        # ================== layer 2 (t-quarter only) ==================
        # ---- g2: stream swap nl<->tl -> z1_fdt [(nh,tl); (c, h, nl)] ----
        z1_fdt = st.tile([128, 2048], bf, tag="z1_fdt")
        z1_t = z1_nm[:].rearrange("p (c h tl) -> p c h tl", c=2, h=32, tl=32)
        z1_fdt_v = z1_fdt[:].rearrange("p (c h nl) -> p c h nl", c=2, h=32, nl=32)
        for c in range(2):
            for k in range(2):
                sl = slice(16 * k, 16 * (k + 1))
                nc.vector.transpose(out=z1_fdt_v[:, c, sl], in_=z1_t[:, c, sl])

        # ---- u2 = At-mix(z1) quarter : psum [(nh,tl'16); (h, nl)] ----
        u2ps = ps.tile([128, 1024], fp, tag="big")
        for hh in range(2):
            dst = u2ps[:, 512 * hh : 512 * (hh + 1)]
            for c in range(2):
                nc.tensor.matmul(
                    dst,
                    cslice("atbq", c),
                    z1_fdt[:, 1024 * c + 512 * hh : 1024 * c + 512 * (hh + 1)],
                    start=(c == 0),
                    stop=(c == 1),
                )

        # ---- u2 -> NM: stream swap tl'<->nl -> [(nh,nl); (h, tl'32)] ----
        u2c = st.tile([128, 1024], bf, tag="u2c")
        nc.scalar.copy(u2c[:], u2ps[:])
        u2_nm = st.tile([128, 1024], bf, tag="u2_nm")
        u2c_v = u2c[:].rearrange("p (h nl) -> p h nl", h=32, nl=32)
        u2_nm_v = u2_nm[:].rearrange("p (h tl) -> p h tl", h=32, tl=32)
        for k in range(2):
            sl = slice(16 * k, 16 * (k + 1))
            nc.vector.transpose(out=u2_nm_v[:, sl], in_=u2c_v[:, sl])

        # ---- zpre2 = P z1q + Q u2 : psum [(n'); (tl', h)], dynamic quarter ----
        zpre2 = ps.tile([128, 512], fp, tag="big")
        z1_tlh = z1_nm[:].rearrange("p (c h tl) -> p c tl h", c=2, h=32, tl=32)
        z1q = z1_tlh[:, ds(cq, 1), ds(tl0, TQ), :]
        nc.tensor.matmul(zpre2[:], cslice("pq", 0), z1q, start=True, stop=False)
        u2q = u2_nm[:].rearrange("p (h tl) -> p tl h", h=32, tl=32)[:, 0:TQ, :]
        nc.tensor.matmul(zpre2[:], cslice("pq", 1), u2q, start=False, stop=True)

        # ---- zf2: DVE cast copy + stream swap nl<->h -> [(nh,h); (tl', nl)] ----
        zf2c = st.tile([128, 512], bf, tag="zf2c")
        nc.vector.tensor_copy(zf2c[:], zpre2[:])
        zf2 = st.tile([128, 512], bf, tag="zf2")
        zf2c_v = zf2c[:].rearrange("p (tl h) -> p tl h", tl=TQ)
        zf2_v = zf2[:].rearrange("p (tl nl) -> p tl nl", tl=TQ)
        nc.vector.transpose(out=zf2_v, in_=zf2c_v)

        # ---- pre2 = zf2 @ Heff2 ; tanh ; out = z2 @ W2 + b2 ----
        pre2 = ps.tile([128, 512], fp, tag="big")
        nc.tensor.matmul(pre2[:], cslice("hi2"), zf2[:], start=True, stop=True)
        h2_fd = st.tile([128, 512], bf, tag="h2_fd")
        nc.scalar.activation(h2_fd[:], pre2[:], AF.Tanh)

        opre = ps.tile([128, 512], fp, tag="big")
        nc.tensor.matmul(opre[:], cslice("w2i4"), h2_fd[:], start=True, stop=True)
        out_s = st.tile([128, 512], bf, tag="out_s")
        nc.scalar.activation(out_s[:], opre[:], AF.Identity, bias=b2t_s)

        nc.sync.dma_start(outb.ap(), out_s[:])

    nc.compile()
    return nc


def _host_weights(Adj_t, Adj_s, s, H, W1, b1, W2, b2):
    import ml_dtypes

    f4, bfd = np.float32, ml_dtypes.bfloat16
    I4 = np.eye(4, dtype=f4)
    I128 = np.eye(128, dtype=f4)
    Heff = H.sum(axis=1).astype(f4)

    P = (s[0] * I128 + s[1] * Adj_s).astype(f4)
    Q = (s[2] * I128 + s[3] * Adj_s).astype(f4)

    W1h = (W1 @ Heff[0]).astype(f4)
    b1h_vec = (b1 @ Heff[0]).astype(f4)
    w1h = np.kron(I4, W1h)
    b1h = np.tile(b1h_vec, 4)[:, None].astype(f4)

    hi2 = np.kron(I4, Heff[1])
    w2pad = np.zeros((32, 32), dtype=f4)
    w2pad[:, :FOUT] = W2
    w2i4 = np.kron(I4, w2pad)
    b2pad = np.zeros(32, dtype=f4)
    b2pad[:FOUT] = b2
    b2t = np.tile(b2pad, 4)[:, None].astype(f4)

    atbd = [
        np.kron(I4, Adj_t[32 * c : 32 * (c + 1), 32 * cp : 32 * (cp + 1)].astype(f4))
        for c in range(2)
        for cp in range(2)
    ]  # index 2c+cp

    atbq_all = []
    for qq in range(NQ):
        blks = []
        for c in range(2):
            blk = np.zeros((32, 32), dtype=f4)
            blk[:, :TQ] = Adj_t[32 * c : 32 * (c + 1), TQ * qq : TQ * (qq + 1)]
            blks.append(np.kron(I4, blk))
        atbq_all.append(blks)

    blobs = []
    for qq in range(NQ):
        blob = np.concatenate(
            [w1h] + atbd + [P, Q] + atbq_all[qq] + [hi2, w2i4], axis=1
        ).astype(bfd)
        assert blob.shape == (128, 1408)
        blobs.append(np.ascontiguousarray(blob))
    bias2 = np.concatenate([b1h, b2t], axis=1).astype(f4)
    return blobs, np.ascontiguousarray(bias2)


def _x_fd(xb):
    """x [8192, 32] f32 -> [(nh,f); (c, tl, nl)] bf16 [128, 2048]."""
    import ml_dtypes

    v = xb.reshape(2, 32, 4, 32, 32)  # [c, tl, nh, nl, f]
    v = v.transpose(2, 4, 0, 1, 3)  # [nh, f, c, tl, nl]
    return np.ascontiguousarray(v.reshape(128, 2048).astype(ml_dtypes.bfloat16))


def _in_maps(inputs):
    f4 = np.float32
    x = np.asarray(inputs["x"], dtype=f4)
    blobs, bias2 = _host_weights(
        np.asarray(inputs["Adj_t"], dtype=f4),
        np.asarray(inputs["Adj_s"], dtype=f4),
        np.asarray(inputs["s"], dtype=f4),
        np.asarray(inputs["H"], dtype=f4),
        np.asarray(inputs["W1"], dtype=f4),
        np.asarray(inputs["b1"], dtype=f4),
        np.asarray(inputs["W2"], dtype=f4),
        np.asarray(inputs["b2"], dtype=f4),
    )
    xfd = [_x_fd(x[b]) for b in range(B)]
    maps = []
    for c in range(NCORES):
        b, qq = c // NQ, c % NQ
        maps.append({"xb": xfd[b], "cblob": blobs[qq], "bias2": bias2})
    return maps


def _gather_out(res_list):
    """Per-core outb [128, 512] bf16 ([(nh,f); (tl', nl)]) -> [B, M, FOUT] f32."""
    out = np.empty((B, M, FOUT), dtype=np.float32)
    for c in range(NCORES):
        b, qq = c // NQ, c % NQ
        o = np.asarray(res_list[c]["outb"]).astype(np.float32)
        v = o.reshape(4, 32, TQ, 32)  # [nh, f, tl, nl]
        v = v.transpose(2, 0, 3, 1).reshape(TQ, 128, 32)  # [tl, n, f]
        tbase = TQ * qq
        out[b, 128 * tbase : 128 * (tbase + TQ), :] = v[:, :, :FOUT].reshape(
            TQ * 128, FOUT
        )
    return out


def kernel(**inputs) -> np.ndarray:
    from concourse import bass_utils

    if "nc" not in _CACHE:
        _CACHE["nc"] = _build_nc()
    nc = _CACHE["nc"]

    maps = _in_maps(inputs)
    import os

    trace = bool(int(os.environ.get("GTCNN_TRACE", "0")))
    res = bass_utils.run_bass_kernel_spmd(
        nc,
        maps,
        core_ids=list(range(NCORES)),
        trace=trace,
        trace_cores=list(range(NCORES)) if trace else None,
        stitch_traces=False,
    )
    _CACHE["last_results"] = res

    return _gather_out(res.results)


# revision 31
# speedup vs baseline: 1.1299x; 1.0691x over previous
"""Trainium2 Bass kernel for nn_GTCNN (product-graph GTCNN, 2 layers, K collapsed).

Math (per batch b, x: [M=8192, 32]):
  Adj = s0*I + s1*kron(I_t, As) + s2*kron(At, I_s) + s3*kron(At, As),  T=64, N=128
  h0 = x @ W1 + b1 ; h_{l+1} = tanh((Adj h_l) @ Heff_l) ; out = h2 @ W2 + b2

Device decomposition (mixes commute: At on t, P/Q on n, Heff on h):
  P = s0*I + s1*As, Q = s2*I + s3*As (symmetric, folded on host)
  L1 (Heff1 commuted into W1): h0' = x @ (W1 Heff1) + b1 Heff1
      z1 = tanh(P h0' + Q (At-mix h0'))
  L2: u2 = At-mix z1 ; z2 = tanh((P z1 + Q u2) @ Heff2) ; out = z2 W2 + b2

Sharding: core c -> (b = c//4, t-quarter q = c%4). L1 computed fully per b.
All data bf16 (PSUM fp32). t = 32c + tl, n = 32nh + nl, h = feature.
Layouts (mem free-order innermost-last); conversions are DVE stream-transposes
(32x32 partition-inner <-> view-inner-32 swaps, baseline-proven at ~859ns/512):
  x/h0_fd  [(nh,h); (c, tl, nl)]
  g1/z1fdt [(nh,tl); (c, h, nl)]   <- stream swap h<->tl (strided views)
  z_nm     [(nh,nl); (c, tl, h)]   <- stream swap h<->nl
  u1ps     [(nh,tl'); (c', h, nl)] -> stream swap tl'<->nl -> u1_nm [(nh,nl); (c',h,tl')]
  u2ps     [(nh,tl'16); (h, nl)]   -> u2_nm [(nh,nl); (h, tl'32: hi half garbage)]
  zf2/out  [(nh,h); (tl', nl)]
"""

import numpy as np

T, NS, B, FIN, HID, FOUT = 64, 128, 2, 32, 32, 16
M = T * NS
NCORES, NQ = 8, 4
TQ = T // NQ

_CACHE = {}

# const blob column offsets (bf16 [128, 1408])
_CO = dict(w1h=0, atbd=128, pq=640, atbq=896, hi2=1152, w2i4=1280)


def _build_nc():
    from contextlib import ExitStack

    import concourse.mybir as mybir
    import concourse.tile as tile
    from concourse import bacc
    from concourse.bass import ds

    fp = mybir.dt.float32
    bf = mybir.dt.bfloat16
    AF = mybir.ActivationFunctionType

    nc = bacc.Bacc(
        "TRN2",
        target_bir_lowering=False,
        debug=False,
        enable_asserts=False,
        num_devices=NCORES,
    )

    xb = nc.dram_tensor("xb", [128, 2048], bf, kind="ExternalInput")
    cblob = nc.dram_tensor("cblob", [128, 1408], bf, kind="ExternalInput")
    bias2 = nc.dram_tensor("bias2", [128, 2], fp, kind="ExternalInput")
    outb = nc.dram_tensor("outb", [128, 512], bf, kind="ExternalOutput")

    C512 = [slice(512 * j, 512 * (j + 1)) for j in range(4)]
    C1024 = [slice(1024 * j, 1024 * (j + 1)) for j in range(2)]

    with tile.TileContext(nc) as tc, ExitStack() as ctx:
        const = ctx.enter_context(tc.tile_pool(name="const", bufs=1))
        st = ctx.enter_context(tc.tile_pool(name="st", bufs=1))
        ps = ctx.enter_context(tc.tile_pool(name="ps", bufs=2, space="PSUM"))

        pid = nc.partition_id()
        q = pid % NQ
        cq = q // 2          # t-half of this core's quarter
        tl0 = (q % 2) * TQ   # tl offset within the half

        cs = const.tile([128, 1408], bf, tag="cblob")
        nc.sync.dma_start(cs[:], cblob.ap())
        bs = const.tile([128, 2], fp, tag="bias2")
        nc.sync.dma_start(bs[:], bias2.ap())

        def cslice(name, k=0):
            o = _CO[name] + 128 * k
            return cs[:, o : o + 128]

        b1h_s = bs[:, 0:1]
        b2t_s = bs[:, 1:2]

        x_s = st.tile([128, 2048], bf, tag="x")
        for j in range(2):
            nc.scalar.dma_start(x_s[:, C1024[j]], xb.ap()[:, C1024[j]])

        # ---- h0' = x @ W1' + b1' : psum/act in (c, tl, nl) order ----
        h0ps = ps.tile([128, 2048], fp, tag="big")
        for j in range(4):
            nc.tensor.matmul(h0ps[:, C512[j]], cslice("w1h"), x_s[:, C512[j]], start=True, stop=True)
        h0_fd = st.tile([128, 2048], bf, tag="h0_fd")
        for j in range(2):
            nc.scalar.activation(h0_fd[:, C1024[j]], h0ps[:, C1024[j]], AF.Identity, bias=b1h_s)

        # ---- g1 first (slow strided group gates the u1 critical path) ----
        g1 = st.tile([128, 2048], bf, tag="g1")
        h0_t = h0_fd[:].rearrange("p (c tl nl) -> p c nl tl", c=2, tl=32, nl=32)
        g1_v = g1[:].rearrange("p (c h nl) -> p c nl h", c=2, h=32, nl=32)
        for c in range(2):
            for k in range(2):
                sl = slice(16 * k, 16 * (k + 1))
                nc.vector.transpose(out=g1_v[:, c, sl], in_=h0_t[:, c, sl])

        # ---- z_nm: stream swap h<->nl -> [(nh,nl); (c, tl, h)] ----
        z_nm = st.tile([128, 2048], bf, tag="z_nm")
        h0_n = h0_fd[:].rearrange("p (c tl nl) -> p c tl nl", c=2, tl=32, nl=32)
        z_nm_v = z_nm[:].rearrange("p (c tl h) -> p c tl h", c=2, tl=32, h=32)
        for c in range(2):
            for k in range(2):
                sl = slice(16 * k, 16 * (k + 1))
                nc.vector.transpose(out=z_nm_v[:, c, sl], in_=h0_n[:, c, sl])

        # ---- zpre1 psum tile (order (c, h, tl)); mms emitted after u1 ----
        zpre1 = ps.tile([128, 2048], fp, tag="big")
        z_nm_q = z_nm[:].rearrange("p (c tl h) -> p c h tl", c=2, tl=32, h=32)

        # ---- u1 = At-mix(h0') : psum [(nh,tl'); (c', h, nl)] ----
        u1ps = ps.tile([128, 2048], fp, tag="big")
        for cp in range(2):
            for hh in range(2):
                dst = u1ps[:, 1024 * cp + 512 * hh : 1024 * cp + 512 * (hh + 1)]
                for c in range(2):
                    nc.tensor.matmul(
                        dst,
                        cslice("atbd", 2 * c + cp),
                        g1[:, 1024 * c + 512 * hh : 1024 * c + 512 * (hh + 1)],
                        start=(c == 0),
                        stop=(c == 1),
                    )

        # ---- u1 -> NM: ScalarE psum->bf16 copy, then stream swap tl'<->nl ----
        u1c = st.tile([128, 2048], bf, tag="u1c")
        for j in range(2):
            nc.scalar.copy(u1c[:, C1024[j]], u1ps[:, C1024[j]])
        u1_nm = st.tile([128, 2048], bf, tag="u1_nm")
        u1c_v = u1c[:].rearrange("p (c h nl) -> p c h nl", c=2, h=32, nl=32)
        u1_nm_v = u1_nm[:].rearrange("p (c h tl) -> p c h tl", c=2, h=32, tl=32)
        for c in range(2):
            for k in range(2):
                sl = slice(16 * k, 16 * (k + 1))
                nc.vector.transpose(out=u1_nm_v[:, c, sl], in_=u1c_v[:, c, sl])

        # ---- zpre1 = P z + Q u1 (P hides in the u1c/stream window; Q last) ----
        for j in range(4):
            mv = z_nm_q[:, j // 2, 16 * (j % 2) : 16 * (j % 2) + 16, :]
            nc.tensor.matmul(zpre1[:, C512[j]], cslice("pq", 0), mv, start=True, stop=False)
        for j in range(4):
            nc.tensor.matmul(zpre1[:, C512[j]], cslice("pq", 1), u1_nm[:, C512[j]], start=False, stop=True)

        # ---- tanh -> z1_nm [(nh,nl); (c, h, tl)] (contiguous) ----
        z1_nm = st.tile([128, 2048], bf, tag="z1_nm")
        for j in range(2):
            nc.scalar.activation(z1_nm[:, C1024[j]], zpre1[:, C1024[j]], AF.Tanh)

        # ================== layer 2 (t-quarter only) ==================
        # ---- g2: stream swap nl<->tl -> z1_fdt [(nh,tl); (c, h, nl)] ----
        z1_fdt = st.tile([128, 2048], bf, tag="z1_fdt")
        z1_t = z1_nm[:].rearrange("p (c h tl) -> p c h tl", c=2, h=32, tl=32)
        z1_fdt_v = z1_fdt[:].rearrange("p (c h nl) -> p c h nl", c=2, h=32, nl=32)
        for c in range(2):
            for k in range(2):
                sl = slice(16 * k, 16 * (k + 1))
                nc.vector.transpose(out=z1_fdt_v[:, c, sl], in_=z1_t[:, c, sl])

        # ---- u2 = At-mix(z1) quarter : psum [(nh,tl'16); (h, nl)] ----
        u2ps = ps.tile([128, 1024], fp, tag="big")
        for hh in range(2):
            dst = u2ps[:, 512 * hh : 512 * (hh + 1)]
            for c in range(2):
                nc.tensor.matmul(
                    dst,
                    cslice("atbq", c),
                    z1_fdt[:, 1024 * c + 512 * hh : 1024 * c + 512 * (hh + 1)],
                    start=(c == 0),
                    stop=(c == 1),
                )

        # ---- u2 -> NM: stream swap tl'<->nl -> [(nh,nl); (h, tl'32)] ----
        u2c = st.tile([128, 1024], bf, tag="u2c")
        nc.scalar.copy(u2c[:], u2ps[:])
        u2_nm = st.tile([128, 1024], bf, tag="u2_nm")
        u2c_v = u2c[:].rearrange("p (h nl) -> p h nl", h=32, nl=32)
        u2_nm_v = u2_nm[:].rearrange("p (h tl) -> p h tl", h=32, tl=32)
        for k in range(2):
            sl = slice(16 * k, 16 * (k + 1))
            nc.vector.transpose(out=u2_nm_v[:, sl], in_=u2c_v[:, sl])

        # ---- zpre2 = P z1q + Q u2 : psum [(n'); (tl', h)], dynamic quarter ----
        zpre2 = ps.tile([128, 512], fp, tag="big")
        z1_tlh = z1_nm[:].rearrange("p (c h tl) -> p c tl h", c=2, h=32, tl=32)
        z1q = z1_tlh[:, ds(cq, 1), ds(tl0, TQ), :]
        nc.tensor.matmul(zpre2[:], cslice("pq", 0), z1q, start=True, stop=False)
        u2q = u2_nm[:].rearrange("p (h tl) -> p tl h", h=32, tl=32)[:, 0:TQ, :]
        nc.tensor.matmul(zpre2[:], cslice("pq", 1), u2q, start=False, stop=True)

        # ---- zf2: DVE cast copy + stream swap nl<->h -> [(nh,h); (tl', nl)] ----
        zf2c = st.tile([128, 512], bf, tag="zf2c")
        nc.vector.tensor_copy(zf2c[:], zpre2[:])
        zf2 = st.tile([128, 512], bf, tag="zf2")
        zf2c_v = zf2c[:].rearrange("p (tl h) -> p tl h", tl=TQ)
        zf2_v = zf2[:].rearrange("p (tl nl) -> p tl nl", tl=TQ)
        nc.vector.transpose(out=zf2_v, in_=zf2c_v)

        # ---- pre2 = zf2 @ Heff2 ; tanh ; out = z2 @ W2 + b2 ----
        pre2 = ps.tile([128, 512], fp, tag="big")
        nc.tensor.matmul(pre2[:], cslice("hi2"), zf2[:], start=True, stop=True)
        h2_fd = st.tile([128, 512], bf, tag="h2_fd")
        nc.scalar.activation(h2_fd[:], pre2[:], AF.Tanh)

        opre = ps.tile([128, 512], fp, tag="big")
        nc.tensor.matmul(opre[:], cslice("w2i4"), h2_fd[:], start=True, stop=True)
        out_s = st.tile([128, 512], bf, tag="out_s")
        nc.scalar.activation(out_s[:], opre[:], AF.Identity, bias=b2t_s)

        nc.sync.dma_start(outb.ap(), out_s[:])

    nc.compile()
    return nc


def _host_weights(Adj_t, Adj_s, s, H, W1, b1, W2, b2):
    import ml_dtypes

    f4, bfd = np.float32, ml_dtypes.bfloat16
    I4 = np.eye(4, dtype=f4)
    I128 = np.eye(128, dtype=f4)
    Heff = H.sum(axis=1).astype(f4)

    P = (s[0] * I128 + s[1] * Adj_s).astype(f4)
    Q = (s[2] * I128 + s[3] * Adj_s).astype(f4)

    W1h = (W1 @ Heff[0]).astype(f4)
    b1h_vec = (b1 @ Heff[0]).astype(f4)
    w1h = np.kron(I4, W1h)
    b1h = np.tile(b1h_vec, 4)[:, None].astype(f4)

    hi2 = np.kron(I4, Heff[1])
    w2pad = np.zeros((32, 32), dtype=f4)
    w2pad[:, :FOUT] = W2
    w2i4 = np.kron(I4, w2pad)
    b2pad = np.zeros(32, dtype=f4)
    b2pad[:FOUT] = b2
    b2t = np.tile(b2pad, 4)[:, None].astype(f4)

    atbd = [
        np.kron(I4, Adj_t[32 * c : 32 * (c + 1), 32 * cp : 32 * (cp + 1)].astype(f4))
        for c in range(2)
        for cp in range(2)
    ]  # index 2c+cp

    atbq_all = []
    for qq in range(NQ):
        blks = []
        for c in range(2):
            blk = np.zeros((32, 32), dtype=f4)
            blk[:, :TQ] = Adj_t[32 * c : 32 * (c + 1), TQ * qq : TQ * (qq + 1)]
            blks.append(np.kron(I4, blk))
        atbq_all.append(blks)

    blobs = []
    for qq in range(NQ):
        blob = np.concatenate(
            [w1h] + atbd + [P, Q] + atbq_all[qq] + [hi2, w2i4], axis=1
        ).astype(bfd)
        assert blob.shape == (128, 1408)
        blobs.append(np.ascontiguousarray(blob))
    bias2 = np.concatenate([b1h, b2t], axis=1).astype(f4)
    return blobs, np.ascontiguousarray(bias2)


def _x_fd(xb):
    """x [8192, 32] f32 -> [(nh,f); (c, tl, nl)] bf16 [128, 2048]."""
    import ml_dtypes

    v = xb.reshape(2, 32, 4, 32, 32)  # [c, tl, nh, nl, f]
    v = v.transpose(2, 4, 0, 1, 3)  # [nh, f, c, tl, nl]
    return np.ascontiguousarray(v.reshape(128, 2048).astype(ml_dtypes.bfloat16))


def _in_maps(inputs):
    f4 = np.float32
    x = np.asarray(inputs["x"], dtype=f4)
    blobs, bias2 = _host_weights(
        np.asarray(inputs["Adj_t"], dtype=f4),
        np.asarray(inputs["Adj_s"], dtype=f4),
        np.asarray(inputs["s"], dtype=f4),
        np.asarray(inputs["H"], dtype=f4),
        np.asarray(inputs["W1"], dtype=f4),
        np.asarray(inputs["b1"], dtype=f4),
        np.asarray(inputs["W2"], dtype=f4),
        np.asarray(inputs["b2"], dtype=f4),
    )
    xfd = [_x_fd(x[b]) for b in range(B)]
    maps = []
    for c in range(NCORES):
        b, qq = c // NQ, c % NQ
        maps.append({"xb": xfd[b], "cblob": blobs[qq], "bias2": bias2})
    return maps


def _gather_out(res_list):
    """Per-core outb [128, 512] bf16 ([(nh,f); (tl', nl)]) -> [B, M, FOUT] f32."""
    out = np.empty((B, M, FOUT), dtype=np.float32)
    for c in range(NCORES):
        b, qq = c // NQ, c % NQ
        o = np.asarray(res_list[c]["outb"]).astype(np.float32)
        v = o.reshape(4, 32, TQ, 32)  # [nh, f, tl, nl]
        v = v.transpose(2, 0, 3, 1).reshape(TQ, 128, 32)  # [tl, n, f]
        tbase = TQ * qq
        out[b, 128 * tbase : 128 * (tbase + TQ), :] = v[:, :, :FOUT].reshape(
            TQ * 128, FOUT
        )
    return out


def kernel(**inputs) -> np.ndarray:
    from concourse import bass_utils

    if "nc" not in _CACHE:
        _CACHE["nc"] = _build_nc()
    nc = _CACHE["nc"]

    maps = _in_maps(inputs)
    import os

    trace = bool(int(os.environ.get("GTCNN_TRACE", "0")))
    res = bass_utils.run_bass_kernel_spmd(
        nc,
        maps,
        core_ids=list(range(NCORES)),
        trace=trace,
        trace_cores=list(range(NCORES)) if trace else None,
        stitch_traces=False,
    )
    _CACHE["last_results"] = res

    return _gather_out(res.results)
